# revision 1
# baseline (speedup 1.0000x reference)
"""
CosmosUnpatcher3d (inverse 3D Haar wavelet, PATCH_SIZE=2) on 8 Trainium2
NeuronCores.

Math: input  x[b, ch, i, j, k] with ch = 3*g + c, g = (bt, bh, bw) bits
      output y[b, c, t, h, w]  with t = 2i+dt, h = 2j+dh, w = 2k+dw
      y = sum_g (-1)^(bt*dt + bh*dh + bw*dw) * x[...]
(the Haar taps (1/sqrt2)^3 times the final sqrt(8) rescale cancel to
exactly 1.0), then the t=0 plane is dropped (17 output t-planes).

This is an 8-point Hadamard transform across the 8 subband planes,
done as a 3-stage butterfly of tensor_tensor add/sub pairs. Per-
instruction overhead dominates on this backend, so the kernel uses
few, large, FLAT-contiguous ops (strided multi-dim APs are slow).

Sharding: 8 cores = batch(2) x H-quarters(4). Each core processes its
(24, 9, 64, 256) shard in 5 triple-buffered rounds with TAPERED sizes
(per-plane elems 324,864,972,864,432 — small edge rounds shrink the
pipeline fill/drain):
  round = [128 partitions, 8 planes x e] resident in SBUF
  in-DMA -> stages 1-2 on VectorE (6 ops)
  -> stage 3 on GPSIMD (8 ops, balances engine load ~50/50)
  -> out-DMA
In- and out-DMAs alternate between the sync and scalar HWDGE queues
each round so both DMA rings stay busy and in/out streams overlap.
Stage-3/z tiles alias the s1 pool so triple buffering fits in SBUF;
CoreSim's TRN2 cost model predicts 73.9 us/core vs the ~77 us naive
DMA roofline (27.7 MB/core at ~358 GB/s). Host packs shards
partition-major (pure data movement; all arithmetic happens on
device) and scatters the 8 result planes into the strided output
positions.
"""

import numpy as np

_N_CORES = 8
_B, _CH, _TI, _HI, _WI = 2, 24, 9, 256, 256
_C_OUT = 3
_JQ = 4               # H-quarter cores per batch entry
_HJ = _HI // _JQ      # 64 input rows per core
_PL = 1728            # per-plane elems per partition (3*9*32*256 / 128)
_F = 8 * _PL          # free-dim elems per partition per round

_cached = {}


def _build_nc(repeat=1):
    import concourse.bacc as bacc
    import concourse.mybir as mybir
    from concourse.tile import TileContext
    from concourse.mybir import AluOpType
    from contextlib import ExitStack

    import os

    f32 = mybir.dt.float32
    add, sub = AluOpType.add, AluOpType.subtract
    nc = bacc.Bacc()

    NBUF = int(os.environ.get("K_BUFS", "3"))
    sizes = _round_sizes()
    TOT = 128 * 8 * sum(sizes)
    X = nc.declare_dram_parameter("x", [TOT], f32, isOutput=False)
    O = nc.declare_dram_parameter("out", [TOT], f32, isOutput=True)

    with TileContext(nc) as tc, ExitStack() as ctx:
        pa = ctx.enter_context(tc.tile_pool(name="pa", bufs=NBUF))
        pb = ctx.enter_context(tc.tile_pool(name="pb", bufs=NBUF))

        for _rep in range(repeat):
            base = 0
            for ri, e in enumerate(sizes):
                FR = 8 * e
                H, Q, E = FR // 2, FR // 4, FR // 8
                blk = 128 * FR
                # alternate HWDGE queues per round: keeps both DMA rings
                # busy so in- and out-streams overlap fully
                in_eng = nc.scalar if ri % 2 == 0 else nc.sync
                out_eng = nc.sync if ri % 2 == 0 else nc.scalar
                t0 = pa.tile([128, FR], f32, tag="a")
                in_eng.dma_start(
                    out=t0[:],
                    in_=X[base : base + blk].rearrange("(p f) -> p f", p=128),
                )
                s1 = pb.tile([128, FR], f32, tag="b")
                # stage 1 (bt -> dt): planes {0..3} vs {4..7} — flat
                nc.vector.tensor_tensor(s1[:, 0:H], t0[:, 0:H], t0[:, H:FR], add)
                nc.vector.tensor_tensor(s1[:, H:FR], t0[:, 0:H], t0[:, H:FR], sub)
                # stage 2 (bh -> dh): within each dt half, {0,1} vs {2,3}
                s2 = pa.tile([128, FR], f32, tag="a")  # reuses t0's slot
                for dt in range(2):
                    b0 = dt * H
                    nc.vector.tensor_tensor(
                        s2[:, b0 : b0 + Q], s1[:, b0 : b0 + Q],
                        s1[:, b0 + Q : b0 + H], add,
                    )
                    nc.vector.tensor_tensor(
                        s2[:, b0 + Q : b0 + H], s1[:, b0 : b0 + Q],
                        s1[:, b0 + Q : b0 + H], sub,
                    )
                # stage 3 (bw -> dw) on GPSIMD: even vs odd within quarters
                z = pb.tile([128, FR], f32, tag="b")
                for qb in range(4):
                    b0 = qb * Q
                    nc.gpsimd.tensor_tensor(
                        z[:, b0 : b0 + E], s2[:, b0 : b0 + E],
                        s2[:, b0 + E : b0 + Q], add,
                    )
                    nc.gpsimd.tensor_tensor(
                        z[:, b0 + E : b0 + Q], s2[:, b0 : b0 + E],
                        s2[:, b0 + E : b0 + Q], sub,
                    )
                out_eng.dma_start(
                    out=O[base : base + blk].rearrange("(p f) -> p f", p=128),
                    in_=z[:],
                )
                base += blk
    nc.finalize()
    return nc


def _get_nc():
    import os

    rep = int(os.environ.get("K_NC_REPEAT", "1"))
    key = ("nc", rep)
    if key not in _cached:
        _cached[key] = _build_nc(rep)
    return _cached[key]


def _round_sizes():
    import os

    spec = os.environ.get("K_TAPER", "324,864,972,864,432")
    sizes = [int(v) for v in spec.split(",")]
    assert sum(sizes) == 2 * _PL, sizes
    return sizes


def _pack_core(xb, jq):
    """xb: (24, 9, 256, 256) one batch entry; -> flat (128*8*3456,) packed."""
    sizes = _round_sizes()
    xs = xb[:, :, jq * _HJ : (jq + 1) * _HJ, :]          # (24, 9, 64, 256)
    v = xs.reshape(8, 3, _TI, 2, 32, 256)                # (g, c, i, jc, jl, k)
    v = v.transpose(3, 1, 2, 4, 5, 0)                    # (jc, c, i, jl, k, g)
    vflat = np.ascontiguousarray(v).reshape(-1, 8)       # (U, g)
    parts = []
    off = 0
    for e in sizes:
        blk = vflat[off : off + 128 * e]                 # (128*e, g)
        parts.append(
            np.ascontiguousarray(
                blk.reshape(128, e, 8).transpose(0, 2, 1)
            ).reshape(-1)
        )
        off += 128 * e
    return np.concatenate(parts)


def kernel(hidden_states: np.ndarray) -> np.ndarray:
    import os
    from concourse.bass_utils import run_bass_kernel_spmd

    x = np.ascontiguousarray(hidden_states, dtype=np.float32)
    assert x.shape == (_B, _CH, _TI, _HI, _WI), x.shape

    nc = _get_nc()
    in_maps = [
        {"x": _pack_core(x[b], jq)} for b in range(_B) for jq in range(_JQ)
    ]
    res = run_bass_kernel_spmd(nc, in_maps, list(range(_N_CORES)))
    _cached["last"] = res

    out = np.empty((_B, _C_OUT, 2 * _TI - 1, 2 * _HI, 2 * _WI), dtype=np.float32)
    tmp = np.empty((_C_OUT, 2 * _TI, 2 * _HJ, 2 * _WI), dtype=np.float32)
    sizes = _round_sizes()
    for ci in range(_N_CORES):
        b, jq = divmod(ci, _JQ)
        o = np.asarray(res.results[ci]["out"]).reshape(-1)   # flat
        yflat = np.empty((8, 2 * 128 * _PL), dtype=np.float32)
        base = 0
        offu = 0
        for e in sizes:
            blk = o[base : base + 128 * 8 * e].reshape(128, 8, e)
            yflat[:, offu : offu + 128 * e] = blk.transpose(1, 0, 2).reshape(
                8, 128 * e
            )
            base += 128 * 8 * e
            offu += 128 * e
        y = yflat.reshape(8, 2, _C_OUT, _TI, 32, 256)    # (slot, jc, c, i, jl, k)
        y = y.transpose(1, 0, 2, 3, 4, 5)                # (jc, slot, c, i, jl, k)
        for jc in range(2):
            for slot in range(8):
                dt, dh, dw = (slot >> 2) & 1, (slot >> 1) & 1, slot & 1
                tmp[
                    :, dt::2, jc * 64 + dh : jc * 64 + 64 : 2, dw::2
                ] = y[jc, slot]
        out[b, :, :, jq * 2 * _HJ : (jq + 1) * 2 * _HJ, :] = tmp[:, 1:]
    return out



# revision 2
# speedup vs baseline: 2.8292x; 2.8292x over previous
"""
CosmosUnpatcher3d (inverse 3D Haar wavelet, PATCH_SIZE=2) on 8 Trainium2
NeuronCores.

Math: input  x[b, ch, i, j, k] with ch = 3*g + c, g = (bt, bh, bw) bits
      output y[b, c, t, h, w]  with t = 2i+dt, h = 2j+dh, w = 2k+dw
      y = sum_g (-1)^(bt*dt + bh*dh + bw*dw) * x[...]
(the Haar taps (1/sqrt2)^3 times the final sqrt(8) rescale cancel to
exactly 1.0), then the t=0 plane is dropped (17 output t-planes).

This is an 8-point Hadamard transform across the 8 subband channels.
The kernel runs it as ONE TensorE matmul per 512-column tile with a
constant 128x128 block-diagonal weight W[(g,r),(d,r')] = H[g,d]*I16
(entries +-1, exact): the whole 3-stage butterfly collapses into the
PE systolic pass, PSUM(f32) accumulates, and DVE evacuates PSUM ->
SBUF with a cast to fp16. All HBM traffic is fp16 (the 2e-2 rel-err
budget dwarfs fp16 rounding, measured ~4e-4), which halves the DMA
bytes vs f32 and leaves the kernel DMA-bound at ~330 GB/s/core:

  per core: in 7.08 MB + out 6.68 MB fp16  ->  ~42 us/core measured
  (pure-DMA floor for the same bytes measured 42.8 us; compute is
  fully hidden: PE ~12 us, DVE copies ~18 us).

Sharding: 8 cores = batch(2) x H-quarters(4), each core owning a
(24, 9, 64, 256) input shard. Layout on device: partition p = g*16 +
(j mod 16); free columns = (i, c, j//16, k) with the i=0 block first.
Round 0 covers i=0 and emits only the dt=1 half (PSUM partitions
64:128 via a [128,64] weight slice) because output t=0 is dropped --
5.6% fewer output bytes. DMA queues are DEDICATED (in-DMAs always on
the scalar HWDGE ring, out-DMAs always on sync): HWDGE rings are FIFO
per issuing engine, so mixing directions on one ring serializes
prefetch behind writeback (measured +8 us on the f32 baseline).

Host packs shards and scatters results (pure data movement; all
arithmetic happens on device).
"""

import numpy as np

_B, _CH, _TI, _HI, _WI = 2, 24, 9, 256, 256
_N_CORES = 8
_JQ = 4                      # H-quarter cores per batch entry
_MM_ROUNDS = (6, 12, 12, 12, 12)   # x512 columns per round; round 0 = i=0
_XT = 128 * 27648            # packed input elems per core (fp16)
_OT = 64 * 3072 + 128 * 24576  # output elems per core (t=0 plane dropped)

_cached = {}


def _hadamard_np():
    w = np.zeros((128, 128), dtype=np.float16)
    for g in range(8):
        for d in range(8):
            s = 1.0 if bin(g & d).count("1") % 2 == 0 else -1.0
            for r in range(16):
                w[g * 16 + r, d * 16 + r] = s
    return w


def _build_nc(loop_n=1, timing=False, unroll=4):
    """Build the Bass module. timing=True swaps the big I/O to internal
    DRAM scratch (dummy contents; timing is data-independent) with a tiny
    external in/out, and wraps the body in a hardware For_i loop so
    device time can be measured by repeat-differencing with negligible
    host-transfer noise."""
    import concourse.bacc as bacc
    import concourse.mybir as mybir
    from concourse.tile import TileContext
    from contextlib import ExitStack

    f16 = mybir.dt.float16
    f32 = mybir.dt.float32
    nc = bacc.Bacc()
    if timing:
        X = nc.dram_tensor("xbuf", [_XT], f16)
        O = nc.dram_tensor("obuf", [_OT], f16)
        xi = nc.declare_dram_parameter("x", [128, 2], f32, isOutput=False)
        oo = nc.declare_dram_parameter("out", [128, 2], f32, isOutput=True)
    else:
        X = nc.declare_dram_parameter("x", [_XT], f16, isOutput=False)
        O = nc.declare_dram_parameter("out", [_OT], f16, isOutput=True)
    W = nc.inline_tensor(_hadamard_np(), name="wmat")

    with TileContext(nc) as tc, ExitStack() as ctx:
        wp = ctx.enter_context(tc.tile_pool(name="wp", bufs=1))
        w_sb = wp.tile([128, 128], f16)
        nc.sync.dma_start(out=w_sb[:], in_=W[:])
        if timing:
            t = wp.tile([128, 2], f32)
            nc.sync.dma_start(out=t[:], in_=xi[:])
            nc.sync.dma_start(out=oo[:], in_=t[:])

        pa = ctx.enter_context(tc.tile_pool(name="pa", bufs=4))
        pb = ctx.enter_context(tc.tile_pool(name="pb", bufs=4))
        psum = ctx.enter_context(tc.tile_pool(name="psum", bufs=8, space="PSUM"))

        def body():
            xbase, obase = 0, 0
            for ri, nch in enumerate(_MM_ROUNDS):
                cw = nch * 512
                p = 64 if ri == 0 else 128
                t0 = pa.tile([128, cw], f16, tag="a")
                nc.scalar.dma_start(
                    out=t0[:],
                    in_=X[xbase : xbase + 128 * cw].rearrange(
                        "(p f) -> p f", p=128
                    ),
                )
                z = pb.tile([p, cw], f16, tag="b")
                lhs = w_sb[:, 64:128] if ri == 0 else w_sb[:]
                for j in range(nch):
                    ps = psum.tile([p, 512], f32, tag="ps")
                    nc.tensor.matmul(
                        ps[:], lhs, t0[:, j * 512 : (j + 1) * 512],
                        start=True, stop=True,
                    )
                    nc.vector.tensor_copy(z[:, j * 512 : (j + 1) * 512], ps[:])
                nc.sync.dma_start(
                    out=O[obase : obase + p * cw].rearrange("(p f) -> p f", p=p),
                    in_=z[:],
                )
                xbase += 128 * cw
                obase += p * cw

        if loop_n > 1:
            assert loop_n % unroll == 0
            with tc.For_i(0, loop_n // unroll, 1):
                for _ in range(unroll):
                    body()
        else:
            body()
    nc.finalize()
    return nc


def _get_nc():
    if "nc" not in _cached:
        _cached["nc"] = _build_nc()
    return _cached["nc"]


def _pack_core(xb, jq):
    """xb: (24, 9, 256, 256) f32 one batch entry -> packed fp16 (_XT,)."""
    xs = xb[:, :, jq * 64 : (jq + 1) * 64, :]              # (24, 9, 64, 256)
    v = xs.reshape(8, 3, 9, 4, 16, 256)                    # (g,c,i,jhi,jlo,k)
    arr = v.transpose(0, 4, 2, 1, 3, 5).astype(np.float16)  # (g,jlo,i,c,jhi,k)
    Xc = arr.reshape(128, 9, 3072)
    parts = [np.ascontiguousarray(Xc[:, 0, :]).reshape(-1)]
    for r in range(1, 5):
        parts.append(
            np.ascontiguousarray(Xc[:, 2 * r - 1 : 2 * r + 1, :]).reshape(-1)
        )
    return np.concatenate(parts)


def _unpack_core(o, out, b, jq):
    """o: (_OT,) fp16 device result -> scatter into out[b,:,:,jq*128:,:]."""
    y0 = o[: 64 * 3072].reshape(4, 16, 3, 4, 256)          # (dlow,jlo,c,jhi,k)
    rest = o[64 * 3072 :]
    tmp = np.empty((3, 18, 128, 512), np.float32)
    tmpv = tmp.reshape(3, 18, 64, 2, 256, 2)
    for r in range(1, 5):
        y = rest[(r - 1) * 128 * 6144 : r * 128 * 6144].reshape(
            8, 16, 2, 3, 4, 256
        )                                                   # (d,jlo,ip,c,jhi,k)
        for d in range(8):
            dt, dh, dw = (d >> 2) & 1, (d >> 1) & 1, d & 1
            yd = y[d].transpose(2, 1, 3, 0, 4).reshape(3, 2, 64, 256)
            tmpv[:, 4 * r - 2 + dt, :, dh, :, dw] = yd[:, 0]
            tmpv[:, 4 * r + dt, :, dh, :, dw] = yd[:, 1]
    for q in range(4):
        dh, dw = (q >> 1) & 1, q & 1
        tmpv[:, 1, :, dh, :, dw] = y0[q].transpose(1, 2, 0, 3).reshape(3, 64, 256)
    out[b, :, :, jq * 128 : (jq + 1) * 128, :] = tmp[:, 1:]


def kernel(hidden_states: np.ndarray) -> np.ndarray:
    from concourse.bass_utils import run_bass_kernel_spmd

    x = np.ascontiguousarray(hidden_states, dtype=np.float32)
    assert x.shape == (_B, _CH, _TI, _HI, _WI), x.shape

    nc = _get_nc()
    in_maps = [{"x": _pack_core(x[b], jq)} for b in range(_B) for jq in range(_JQ)]
    res = run_bass_kernel_spmd(nc, in_maps, list(range(_N_CORES)))
    _cached["last"] = res

    out = np.empty((_B, 3, 2 * _TI - 1, 2 * _HI, 2 * _WI), dtype=np.float32)
    for ci in range(_N_CORES):
        b, jq = divmod(ci, _JQ)
        o = np.asarray(res.results[ci]["out"]).reshape(-1)
        _unpack_core(o, out, b, jq)
    return out


# revision 13
# speedup vs baseline: 2.9380x; 1.0385x over previous
"""
CosmosUnpatcher3d (inverse 3D Haar wavelet, PATCH_SIZE=2) on 8 Trainium2
NeuronCores.

Math: input  x[b, ch, i, j, k] with ch = 3*g + c, g = (bt, bh, bw) bits
      output y[b, c, t, h, w]  with t = 2i+dt, h = 2j+dh, w = 2k+dw
      y = sum_g (-1)^(bt*dt + bh*dh + bw*dw) * x[...]
(the Haar taps (1/sqrt2)^3 times the final sqrt(8) rescale cancel to
exactly 1.0), then the t=0 plane is dropped (17 output t-planes).

This is an 8-point Hadamard transform across the 8 subband channels.
The kernel runs it as ONE TensorE matmul per 512-column tile with a
constant 128x128 block-diagonal weight W[(g,r),(d,r')] = H[g,d]*I16
(entries +-1, exact): the whole 3-stage butterfly collapses into the
PE systolic pass, PSUM(f32) accumulates, and DVE evacuates PSUM ->
SBUF with a cast to fp16. All HBM traffic is fp16 (the 2e-2 rel-err
budget dwarfs fp16 rounding, measured ~4e-4), which halves the DMA
bytes vs f32 and leaves the kernel DMA-bound at ~330 GB/s/core:

  per core: in 7.08 MB + out 6.68 MB fp16  ->  ~41 us/core measured
  (pure-DMA floor measured for the same byte count ~42 us; compute is
  fully hidden: PE ~12 us, DVE copies ~18 us).

Sharding: 8 cores = batch(2) x H-quarters(4), each core owning a
(24, 9, 64, 256) input shard. Layout on device: partition p = g*16 +
(j mod 16); free columns = (i, c, j//16, k). The i=0 block is packed
as the LAST round, which emits only the dt=1 half (a [128,64] weight
slice -> 64 PSUM partitions) because output t=0 is dropped -- 5.6%
fewer output bytes, and its small out-DMA shortens the pipeline drain.
DMA queues are DEDICATED (in-DMAs always on the scalar HWDGE ring,
out-DMAs always on sync): HWDGE rings are FIFO per issuing engine, so
mixing directions on one ring serializes prefetch behind writeback
(measured +8 us on the f32 baseline).

Host packs shards and scatters results (pure data movement; all
arithmetic happens on device).
"""

import numpy as np

_B, _CH, _TI, _HI, _WI = 2, 24, 9, 256, 256
_N_CORES = 8
_JQ = 4                      # H-quarter cores per batch entry
_MM_ROUNDS = (8, 8, 8, 8, 8, 8, 6)  # x512 columns per round
_I0_POS = 6                  # index of the i=0 round (64 out partitions)
_XT = 128 * 27648            # packed input elems per core (fp16)
_OT = 64 * 3072 + 128 * 24576  # output elems per core (t=0 plane dropped)

_cached = {}


def _hadamard_np():
    w = np.zeros((128, 128), dtype=np.float16)
    for g in range(8):
        for d in range(8):
            s = 1.0 if bin(g & d).count("1") % 2 == 0 else -1.0
            for r in range(16):
                w[g * 16 + r, d * 16 + r] = s
    return w


def _build_nc(loop_n=1, timing=False, unroll=4):
    """Build the Bass module. timing=True swaps the big I/O to internal
    DRAM scratch (dummy contents; timing is data-independent) with a tiny
    external in/out, and wraps the body in a hardware For_i loop so
    device time can be measured by repeat-differencing with negligible
    host-transfer noise."""
    import concourse.bacc as bacc
    import concourse.mybir as mybir
    from concourse.tile import TileContext
    from contextlib import ExitStack

    f16 = mybir.dt.float16
    f32 = mybir.dt.float32
    nc = bacc.Bacc()
    if timing:
        X = nc.dram_tensor("xbuf", [_XT], f16)
        O = nc.dram_tensor("obuf", [_OT], f16)
        xi = nc.declare_dram_parameter("x", [128, 2], f32, isOutput=False)
        oo = nc.declare_dram_parameter("out", [128, 2], f32, isOutput=True)
    else:
        X = nc.declare_dram_parameter("x", [_XT], f16, isOutput=False)
        O = nc.declare_dram_parameter("out", [_OT], f16, isOutput=True)
    W = nc.inline_tensor(_hadamard_np(), name="wmat")

    with TileContext(nc) as tc, ExitStack() as ctx:
        wp = ctx.enter_context(tc.tile_pool(name="wp", bufs=1))
        w_sb = wp.tile([128, 128], f16)
        nc.sync.dma_start(out=w_sb[:], in_=W[:])
        if timing:
            t = wp.tile([128, 2], f32)
            nc.sync.dma_start(out=t[:], in_=xi[:])
            nc.sync.dma_start(out=oo[:], in_=t[:])

        pa = ctx.enter_context(tc.tile_pool(name="pa", bufs=6))
        pb = ctx.enter_context(tc.tile_pool(name="pb", bufs=6))
        psum = ctx.enter_context(tc.tile_pool(name="psum", bufs=8, space="PSUM"))

        def body():
            xbase, obase = 0, 0
            for ri, nch in enumerate(_MM_ROUNDS):
                cw = nch * 512
                p = 64 if ri == _I0_POS else 128
                t0 = pa.tile([128, cw], f16, tag="a")
                nc.scalar.dma_start(
                    out=t0[:],
                    in_=X[xbase : xbase + 128 * cw].rearrange(
                        "(p f) -> p f", p=128
                    ),
                )
                z = pb.tile([p, cw], f16, tag="b")
                lhs = w_sb[:, 64:128] if ri == _I0_POS else w_sb[:]
                for j in range(nch):
                    ps = psum.tile([p, 512], f32, tag="ps")
                    nc.tensor.matmul(
                        ps[:], lhs, t0[:, j * 512 : (j + 1) * 512],
                        start=True, stop=True,
                    )
                    nc.vector.tensor_copy(z[:, j * 512 : (j + 1) * 512], ps[:])
                nc.sync.dma_start(
                    out=O[obase : obase + p * cw].rearrange("(p f) -> p f", p=p),
                    in_=z[:],
                )
                xbase += 128 * cw
                obase += p * cw

        if loop_n > 1:
            assert loop_n % unroll == 0
            with tc.For_i(0, loop_n // unroll, 1):
                for _ in range(unroll):
                    body()
        else:
            body()
    nc.finalize()
    return nc


def _get_nc():
    if "nc" not in _cached:
        _cached["nc"] = _build_nc()
    return _cached["nc"]


def _pack_core(xb, jq):
    """xb: (24, 9, 256, 256) f32 one batch entry -> packed fp16 (_XT,)."""
    xs = xb[:, :, jq * 64 : (jq + 1) * 64, :]              # (24, 9, 64, 256)
    v = xs.reshape(8, 3, 9, 4, 16, 256)                    # (g,c,i,jhi,jlo,k)
    arr = v.transpose(0, 4, 2, 1, 3, 5).astype(np.float16)  # (g,jlo,i,c,jhi,k)
    Xc = arr.reshape(128, 9, 3072)
    rest = np.ascontiguousarray(Xc[:, 1:, :]).reshape(128, 24576)
    parts = []
    off = 0
    for ri, nch in enumerate(_MM_ROUNDS):
        cw = nch * 512
        if ri == _I0_POS:
            parts.append(np.ascontiguousarray(Xc[:, 0, :]).reshape(-1))
        else:
            parts.append(np.ascontiguousarray(rest[:, off : off + cw]).reshape(-1))
            off += cw
    return np.concatenate(parts)


def _unpack_core(o, out, b, jq):
    """o: (_OT,) fp16 device result -> scatter into out[b,:,:,jq*128:,:]."""
    obig = np.empty((128, 24576), np.float16)
    y0 = None
    pos, off = 0, 0
    for ri, nch in enumerate(_MM_ROUNDS):
        cw = nch * 512
        if ri == _I0_POS:
            y0 = o[pos : pos + 64 * cw].reshape(4, 16, 3, 4, 256)  # (dlow,jlo,c,jhi,k)
            pos += 64 * cw
        else:
            obig[:, off : off + cw] = o[pos : pos + 128 * cw].reshape(128, cw)
            pos += 128 * cw
            off += cw
    tmp = np.empty((3, 18, 128, 512), np.float32)
    tmpv = tmp.reshape(3, 18, 64, 2, 256, 2)
    y = obig.reshape(8, 16, 8, 3, 4, 256)                  # (d,jlo,i8,c,jhi,k)
    for d in range(8):
        dt, dh, dw = (d >> 2) & 1, (d >> 1) & 1, d & 1
        yd = y[d].transpose(2, 1, 3, 0, 4).reshape(3, 8, 64, 256)
        tmpv[:, 2 + dt :: 2, :, dh, :, dw] = yd
    for q in range(4):
        dh, dw = (q >> 1) & 1, q & 1
        tmpv[:, 1, :, dh, :, dw] = y0[q].transpose(1, 2, 0, 3).reshape(3, 64, 256)
    out[b, :, :, jq * 128 : (jq + 1) * 128, :] = tmp[:, 1:]


def kernel(hidden_states: np.ndarray) -> np.ndarray:
    from concourse.bass_utils import run_bass_kernel_spmd

    x = np.ascontiguousarray(hidden_states, dtype=np.float32)
    assert x.shape == (_B, _CH, _TI, _HI, _WI), x.shape

    nc = _get_nc()
    in_maps = [{"x": _pack_core(x[b], jq)} for b in range(_B) for jq in range(_JQ)]
    res = run_bass_kernel_spmd(nc, in_maps, list(range(_N_CORES)))
    _cached["last"] = res

    out = np.empty((_B, 3, 2 * _TI - 1, 2 * _HI, 2 * _WI), dtype=np.float32)
    for ci in range(_N_CORES):
        b, jq = divmod(ci, _JQ)
        o = np.asarray(res.results[ci]["out"]).reshape(-1)
        _unpack_core(o, out, b, jq)
    return out
